# revision 64
# baseline (speedup 1.0000x reference)
"""LowRankAttention Trainium2 kernel (v3: head-pair-split DMA + bf16 stage A).

Math shortcut: scores = Q K^T / 8 per (batch, head) has rank <= d_head = 64,
while the truncated SVD keeps rank min(int(1024*0.1), 256) = 102 > 64, so the
low-rank reconstruction is EXACT and the module reduces to plain softmax
attention. Scores are ~N(0,1) (|s| < 8 for these inputs), so exp without
max-subtraction is fp32-safe; the softmax denominator comes for free from a
ones-column appended to the V weights of the PV matmul.

Sharding: 32 (batch, head) pairs over 8 cores; core c owns batch c//4 and
heads 4*(c%4) .. +4 (d_model cols 256*(c%4) .. +256). No collectives.

Structure (per core; all costs measured from neuron-profile ntff):
- The host splits inputs into per-head-pair [1024,128] tensors. Nine
  loads are chained FIFO on the sync HWDGE ring in consumption order
  (q0 k0a k0b1 v0a k0b2 v0b k1 q1 v1), each in (p t) layout = one
  2-4KB-contiguous descriptor per partition (~21ns/descriptor HWDGE
  processing makes small descriptors the dominant cost otherwise). k0's
  tile-0 slot rides its own tiny DMA so the first scores matmul is not
  gated on a 512KB completion; head-pair-1 data lands ~14 steps before
  it is needed.
- All Q/K transposes ride the bf16 path (DVE pre-cast + 1-cycle/row PE
  transpose). PE HAM warm-up matmuls fill the pre-DMA window (transposes
  don't count as PE-busy for the clock gate).
- Steady state: per kt step the PE runs one row-packed scores pair + two
  lag-2 PV matmuls under a gapless ~1.0us/step ACT exp stream; stage A/C
  work rides a paced filler deque. DVE completion sems are counters, so
  any DVE op waiting on a late DMA gates every later-emitted PE filler:
  DMA-gated casts are placed so they pop only after their data lands,
  and pops are skipped at each group boundary's kt0 so the accumulator
  drains clear the DVE promptly. Three PSUM acc banks (pss 1 + sc 4 +
  ac 3 = 8) give each group's first PV a bank whose drain finished a
  full drain-length before the PV issues.
- Outputs are per-(head-pair, q-chunk) [128,4,128] buffers -> every
  store is 128 x 2KB-contiguous descriptors, fired as soon as its
  group's stage C completes; three stores drain mid-stream. The final
  step runs exp/PV/drain/normalize/store in q-halves across both HWDGE
  rings so the last store issues ~1.7us earlier (the ~3us HBM write
  receipt after the last byte is the kernel's true tail).
"""

import sys

if "/opt/trn_rl_repo" not in sys.path:
    sys.path.insert(0, "/opt/trn_rl_repo")

from collections import deque
from contextlib import ExitStack

import numpy as np

import concourse.bass as bass
import concourse.bacc as bacc
import concourse.tile as tile
from concourse import mybir
from concourse.masks import make_identity
from concourse.bass_utils import run_bass_kernel_spmd

B, S, D = 2, 1024, 1024
H, DH = 16, 64
N_CORES = 8
HPC = 4          # heads per core
CW = HPC * DH    # per-core column width = 256
HW = 128         # head-pair width
FP32 = mybir.dt.float32
BF16 = mybir.dt.bfloat16
EXPF = mybir.ActivationFunctionType.Exp
NKT = 8          # k tiles of 128
N_WARM = 8       # PE HAM warm-up matmuls during the initial DMA wait

_CACHE: dict = {}


def _build_program() -> bass.Bass:
    nc = bacc.Bacc(trn_type="TRN2", num_swdge_queues=1)
    ins = {}
    for hp in range(2):
        for nm in ("q", "k", "v"):
            ins[(nm, hp)] = nc.dram_tensor(f"{nm}{hp}", [S, HW], FP32,
                                           kind="ExternalInput")
    o_d = {hp: nc.dram_tensor(f"o{hp}", [S, HW], FP32, kind="ExternalOutput")
           for hp in range(2)}

    with ExitStack() as ctx:
        tc = ctx.enter_context(tile.TileContext(nc))
        const = ctx.enter_context(tc.tile_pool(name="const", bufs=1))
        raw_p = ctx.enter_context(tc.tile_pool(name="raw", bufs=6))
        natb = ctx.enter_context(tc.tile_pool(name="natb", bufs=4))
        vo_p = ctx.enter_context(tc.tile_pool(name="vo", bufs=4))
        qt_p = ctx.enter_context(tc.tile_pool(name="qt", bufs=2))
        kt_p = ctx.enter_context(tc.tile_pool(name="kt", bufs=2))
        pt_p = ctx.enter_context(tc.tile_pool(name="pt", bufs=4))
        ot_p = ctx.enter_context(tc.tile_pool(name="ot", bufs=4))
        os_p = ctx.enter_context(tc.tile_pool(name="os", bufs=4))
        rc_p = ctx.enter_context(tc.tile_pool(name="rc", bufs=4))
        idb_p = ctx.enter_context(tc.tile_pool(name="idb", bufs=1))
        # PSUM budget: pss 1 + sc 2x2 + ac 3x1 = 8 banks. Three acc banks
        # make each group's first PV land on a bank whose drain finished a
        # full drain-length before the PV issues (no boundary stall).
        pss = ctx.enter_context(tc.tile_pool(name="pss", bufs=1, space="PSUM"))
        sc_p = ctx.enter_context(tc.tile_pool(name="sc", bufs=2, space="PSUM"))
        ac_p = ctx.enter_context(tc.tile_pool(name="ac", bufs=3, space="PSUM"))

        # ---------- input DMA chain: (p t) layout, 4KB/partition
        # descriptors, chained FIFO on sync in consumption order. k0's
        # tile-0 slot rides its own tiny DMA so the critical first scores
        # matmul isn't gated on the full 512KB k0 completion. ----------
        raws = {}

        def load(nm, hp, t0, nt, tag):
            rw = raw_p.tile([128, nt, HW], FP32, tag="raw", name=tag)
            nc.sync.dma_start(
                out=rw,
                in_=ins[(nm, hp)].rearrange("(p t) c -> p t c",
                                            p=128)[:, t0:t0 + nt, :],
            )
            raws[tag] = rw

        load("q", 0, 0, NKT, "q0")
        load("k", 0, 0, 1, "k0a")
        load("k", 0, 1, 3, "k0b1")
        load("v", 0, 0, 2, "v0a")
        load("k", 0, 4, 4, "k0b2")
        load("v", 0, 2, 6, "v0b")
        load("k", 1, 0, NKT, "k1")
        load("q", 1, 0, NKT, "q1")
        load("v", 1, 0, NKT, "v1")

        identb = idb_p.tile([128, 128], BF16)
        make_identity(nc, identb)
        ident = const.tile([128, 128], FP32)
        make_identity(nc, ident)
        # the exp table set loads via the pseudo-instruction walrus attaches
        # to ACT's first activation (the prologue scalar.copy casts below),
        # firing immediately at kernel start under the q0 DMA drain
        # HAM warm-up: real matmuls (transposes don't count as PE-busy)
        wrm = const.tile([128, 512], BF16)
        nc.vector.memset(wrm, 0.0)
        wps = ac_p.tile([128, 512], FP32, tag="ac", name="warmps")
        for _ in range(N_WARM):
            nc.tensor.matmul(wps, lhsT=identb, rhs=wrm, start=True, stop=True)

        # KT/QT: two heads stacked on partitions 0:64 / 64:128 so the d=64
        # scores matmuls row-pack into the PE array as concurrent pairs.
        kt_sb = {hp: kt_p.tile([128, S], BF16, tag="kt", name=f"kt{hp}")
                 for hp in range(2)}
        qt_sb = {hp: qt_p.tile([128, S], BF16, tag="qt", name=f"qt{hp}")
                 for hp in range(2)}

        # split raw tiles: map (nm, hp) -> [(tag, slot0, nslots), ...]
        pieces = {
            ("k", 0): [("k0a", 0, 1), ("k0b1", 1, 3), ("k0b2", 4, 4)],
            ("v", 0): [("v0a", 0, 2), ("v0b", 2, 6)],
            ("q", 0): [("q0", 0, 8)],
            ("k", 1): [("k1", 0, 8)],
            ("q", 1): [("q1", 0, 8)],
            ("v", 1): [("v1", 0, 8)],
        }

        def raw_view(nm, hp, j0, nj):
            """AP view of raw slots j0..j0+nj-1 (must lie in one piece)."""
            for tag, s0, ns in pieces[(nm, hp)]:
                if s0 <= j0 and j0 + nj <= s0 + ns:
                    return raws[tag][:, j0 - s0:j0 - s0 + nj, :]
            raise AssertionError((nm, hp, j0, nj))

        def cast_raw(nm, hp, j0=0, nj=NKT, eng=None):
            """Pre-cast of raw fp32 -> bf16 (tile slots j0:j0+nj). Stream
            casts ride the otherwise-idle GpSimd engine so a cast waiting
            on a late DMA never gates DVE's completion counter."""
            key = ("nb", nm, hp)
            if key not in _CACHE:
                _CACHE[key] = natb.tile([128, NKT, HW], BF16, tag="natb",
                                        name=f"nb{nm}{hp}")
            eng = eng if eng is not None else nc.vector
            if eng is nc.scalar:
                nc.scalar.copy(out=_CACHE[key][:, j0:j0 + nj, :],
                               in_=raw_view(nm, hp, j0, nj))
            else:
                eng.tensor_copy(out=_CACHE[key][:, j0:j0 + nj, :],
                                in_=raw_view(nm, hp, j0, nj))

        def tp_bf16(nm, hp, dst, j0, nj, pool=None):
            """bf16 PE transposes of tiles j0..j0+nj-1 -> dst[:, 128*j]."""
            nb = _CACHE[("nb", nm, hp)]
            pool = pool if pool is not None else pss
            ps = pool.tile([128, 128 * nj], BF16, tag=pool is pss and "pss"
                           or "ac", name=f"ps{nm}{hp}{j0}")
            for j in range(nj):
                nc.tensor.transpose(
                    out=ps[:, j * 128:(j + 1) * 128],
                    in_=nb[:, j0 + j, :], identity=identb)
            nc.vector.tensor_copy(
                out=dst[:, j0 * 128:(j0 + nj) * 128], in_=ps)

        # V weights [V|1]: no transpose needed (V is naturally k-major)
        vos = {}
        for hp in range(2):
            for hi in range(2):
                vo = vo_p.tile([128, NKT, DH + 1], BF16, tag="vo",
                               name=f"vo{hp}{hi}")
                nc.vector.memset(vo[:, :, DH:DH + 1], 1.0)
                vos[(hp, hi)] = vo

        def vo_cast(hp, hi, s0=0, ns=NKT):
            nc.vector.tensor_copy(
                out=vos[(hp, hi)][:, s0:s0 + ns, 0:DH],
                in_=raw_view("v", hp, s0, ns)[:, :, hi * DH:(hi + 1) * DH])

        # output: per-(hp,qc) buffers -> every store is 128 x 2KB-contiguous
        # descriptors and fires as soon as its group's stage C completes
        osb = {(hp, qc): os_p.tile([128, 4, HW], FP32, tag="os",
                                   name=f"osb{hp}{qc}")
               for hp in range(2) for qc in range(2)}
        o_v = {hp: o_d[hp].rearrange("(p t) c -> p t c", p=128)
               for hp in range(2)}

        # ---------- prologue stage A (critical path to the first exp):
        # fine-grained q0 chains (2 tiles per cast/tp/copy hop) while k0's
        # tiny j0 load lands, then the k0 j0 tile ----
        cast_raw("q", 0, 0, 2, eng=nc.vector)
        tp_bf16("q", 0, qt_sb[0], 0, 2)
        # ACT is idle after its table load: run the second q cast and the
        # k0 tile-0 cast there, concurrent with DVE's first cast + copies
        cast_raw("q", 0, 2, 2, eng=nc.scalar)
        tp_bf16("q", 0, qt_sb[0], 2, 2)
        nc.tensor.matmul(wps[:, 0:128], lhsT=identb, rhs=wrm[:, 0:128],
                         start=True, stop=True)
        cast_raw("k", 0, 0, 1, eng=nc.scalar)
        tp_bf16("k", 0, kt_sb[0], 0, 1, pool=ac_p)

        # ---------- filler deque (popped ~1 per kt step). DVE completion
        # sems are counters, so a DVE op waiting on a late DMA gates every
        # later-emitted PE filler: hp1 casts sit in group-1 slots (their
        # DMAs land during group 0) with no-op pads before the v1 casts.
        def ham_warm(n):
            # tiny real matmul: transposes don't count as PE-busy for the
            # HAM clock gate, so trickle MMs through the stage-A window to
            # keep the PE at 2.4GHz until the scores/PV stream sustains it
            nc.tensor.matmul(wps[:, 0:n], lhsT=identb, rhs=wrm[:, 0:n],
                             start=True, stop=True)

        fillers: deque = deque()
        fillers.append(lambda: (cast_raw("k", 0, 1, 3, eng=nc.vector),
                                tp_bf16("k", 0, kt_sb[0], 1, 3),
                                ham_warm(256)))                     # g0k0
        fillers.append(lambda: (vo_cast(0, 0, 0, 2),
                                vo_cast(0, 1, 0, 2),
                                ham_warm(256)))                     # g0k1
        fillers.append(lambda: (vo_cast(0, 0, 2, 2),
                                vo_cast(0, 1, 2, 2)))               # g0k2
        fillers.append(lambda: (cast_raw("k", 0, 4, 4),
                                tp_bf16("k", 0, kt_sb[0], 4, 4)))   # g0k3
        fillers.append(lambda: (vo_cast(0, 0, 4, 4),
                                vo_cast(0, 1, 4, 4)))               # g0k4
        fillers.append(lambda: (cast_raw("q", 0, 4, 4),
                                tp_bf16("q", 0, qt_sb[0], 4, 4)))   # g0k5
        fillers.append(lambda: vo_cast(1, 0))                       # g0k6
        fillers.append(lambda: vo_cast(1, 1))                       # g0k7
        fillers.append(lambda: cast_raw("k", 1))                    # g1k2
        fillers.append(lambda: tp_bf16("k", 1, kt_sb[1], 0, 4))     # g1k3
        fillers.append(lambda: tp_bf16("k", 1, kt_sb[1], 4, 4))     # g1k4
        fillers.append(lambda: cast_raw("q", 1))                    # g1k5
        fillers.append(lambda: tp_bf16("q", 1, qt_sb[1], 0, 4))     # g1k6
        fillers.append(lambda: tp_bf16("q", 1, qt_sb[1], 4, 4))     # g1k7

        def emit_acc_drain(hp, qc, accs, last=False):
            """PSUM->SBUF copies freeing the accumulator banks, emitted
            eagerly when a group's last PV retires. For the final group ACT
            is idle, so one copy runs there concurrently."""
            if last:
                # tail: everything is emitted at once, so pipeline in
                # q-halves: drain a-halves first (ACT: hi0, DVE: hi1) so
                # transposes/normalize/store of the a-half overlap the
                # b-half drains
                for hi in range(2):
                    oT = ot_p.tile([DH + 1, 512], FP32, tag="ot",
                                   name=f"oTl{hi}")
                    _CACHE[("c", hp, qc, hi)] = oT
                for half in range(2):
                    sl = slice(half * 256, (half + 1) * 256)
                    nc.scalar.copy(out=_CACHE[("c", hp, qc, 0)][:, sl],
                                   in_=accs[0][:, sl])
                    nc.vector.tensor_copy(
                        out=_CACHE[("c", hp, qc, 1)][:, sl],
                        in_=accs[1][:, sl])
                return
            for hi in range(2):
                oT = ot_p.tile([DH + 1, 512], FP32, tag="ot",
                               name=f"oT{hp}{qc}{hi}")
                _CACHE[("c", hp, qc, hi)] = oT
                nc.vector.tensor_copy(out=oT, in_=accs[hi])

        def tail_stage_c(hp, qc):
            """Last group: half-pipelined stage C so the first store issues
            while the b-half still normalizes."""
            trs = {}
            trs[0] = pss.tile([128, 4, DH + 1], FP32, tag="pss", name="trl0")
            trs[1] = ac_p.tile([128, 4, DH + 1], FP32, tag="ac", name="trl1")
            dst = o_v[hp][:, qc * 4:(qc + 1) * 4, :]
            for half in range(2):
                for hi in range(2):
                    oT = _CACHE[("c", hp, qc, hi)]
                    for qt in range(half * 2, half * 2 + 2):
                        nc.tensor.transpose(
                            out=trs[hi][:, qt, :],
                            in_=oT[:, qt * 128:(qt + 1) * 128],
                            identity=ident[0:DH + 1, 0:DH + 1])
                for hi in range(2):
                    tr = trs[hi]
                    sl = slice(half * 2, half * 2 + 2)
                    r2 = rc_p.tile([128, 2], FP32, tag="rc",
                                   name=f"r2l{half}{hi}")
                    nc.vector.reciprocal(out=r2, in_=tr[:, sl, DH:DH + 1])
                    r2b = bass.AP(tensor=r2.tensor, offset=r2.offset,
                                  ap=[r2.ap[0], [1, 2], [0, DH]])
                    nc.vector.tensor_tensor(
                        out=osb[(hp, qc)][:, sl, hi * DH:(hi + 1) * DH],
                        in0=tr[:, sl, 0:DH], in1=r2b,
                        op=mybir.AluOpType.mult)
                eng = nc.scalar if half == 0 else nc.sync
                eng.dma_start(out=dst[:, half * 2:half * 2 + 2, :],
                              in_=osb[(hp, qc)][:, half * 2:half * 2 + 2, :])

        def stage_c_steps(hp, qc, last=False):
            """Per hi: [2 transposes (tr alloc)], [2 transposes], [normalize
            + maybe store]. tr tiles are allocated inside the closures so the
            pss pool rotation matches pop order."""
            if last:
                return [lambda: tail_stage_c(hp, qc)]
            steps = []

            def c_tp_a(hi):
                oT = _CACHE[("c", hp, qc, hi)]
                # tail: hi1's transpose PSUM comes from the (drained) acc
                # pool so it doesn't serialize behind hi0's in the 1-buf
                # pss pool
                pool, tg = (ac_p, "ac") if (last and hi == 1) else (pss,
                                                                    "pss")
                tr = pool.tile([128, 4, DH + 1], FP32, tag=tg,
                               name=f"tr{hp}{qc}{hi}")
                _CACHE[("tr", hp, qc, hi)] = tr
                for qt in range(2):
                    nc.tensor.transpose(
                        out=tr[:, qt, :],
                        in_=oT[:, qt * 128:(qt + 1) * 128],
                        identity=ident[0:DH + 1, 0:DH + 1])

            def c_tp_b(hi):
                oT = _CACHE[("c", hp, qc, hi)]
                tr = _CACHE[("tr", hp, qc, hi)]
                for qt in range(2, 4):
                    nc.tensor.transpose(
                        out=tr[:, qt, :],
                        in_=oT[:, qt * 128:(qt + 1) * 128],
                        identity=ident[0:DH + 1, 0:DH + 1])

            def c_norm(hi):
                tr = _CACHE[("tr", hp, qc, hi)]
                r4 = rc_p.tile([128, 4], FP32, tag="rc",
                               name=f"r4{hp}{qc}{hi}")
                nc.vector.reciprocal(out=r4, in_=tr[:, :, DH:DH + 1])
                r4b = bass.AP(tensor=r4.tensor, offset=r4.offset,
                              ap=[r4.ap[0], [1, 4], [0, DH]])
                nc.vector.tensor_tensor(
                    out=osb[(hp, qc)][:, :, hi * DH:(hi + 1) * DH],
                    in0=tr[:, :, 0:DH], in1=r4b, op=mybir.AluOpType.mult)
                if hi == 1:
                    dst = o_v[hp][:, qc * 4:(qc + 1) * 4, :]
                    src = osb[(hp, qc)]
                    if last:
                        # tail: split across both HWDGE rings
                        nc.scalar.dma_start(out=dst[:, 0:2, :],
                                            in_=src[:, 0:2, :])
                        nc.sync.dma_start(out=dst[:, 2:4, :],
                                          in_=src[:, 2:4, :])
                    else:
                        nc.sync.dma_start(out=dst, in_=src)

            for hi in range(2):
                steps.append(lambda hi=hi: c_tp_a(hi))
                steps.append(lambda hi=hi: c_tp_b(hi))
                steps.append(lambda hi=hi: c_norm(hi))
            return steps

        # flat 32-step pipeline: PV trails the exp stream by TWO steps and
        # crosses group boundaries, so the in-order PE chain always runs a
        # full step ahead of the ACT exp stream.
        groups = [(0, 0), (0, 1), (1, 0), (1, 1)]  # (hp, qc)
        pend = deque()  # (pt, kt, accs, hp, qc, last)

        def flush_pv(entry, half=None):
            ppt, pkt, accs_, hp_, qc_, last_ = entry
            lo, w = (0, 512) if half is None else (half * 256, 256)
            for hi in range(2):
                nc.tensor.matmul(
                    accs_[hi][:, lo:lo + w],
                    lhsT=vos[(hp_, hi)][:, pkt, :],
                    rhs=ppt[:, hi * 512 + lo:hi * 512 + lo + w],
                    start=(pkt == 0), stop=(pkt == NKT - 1),
                )
            if pkt == NKT - 1 and (half is None or half == 1):
                emit_acc_drain(hp_, qc_, accs_, last=last_)
                fillers.extend(stage_c_steps(hp_, qc_, last=last_))

        for gi, (hp, qc) in enumerate(groups):
            accs = [ac_p.tile([DH + 1, 512], FP32, tag="ac",
                              name=f"acc{hp}_{qc}_{i}") for i in range(2)]
            for kt in range(NKT):
                sc = sc_p.tile([128, 1024], FP32, tag="sc",
                               name=f"sc{gi}_{kt}")
                for hi in range(2):
                    nc.tensor.matmul(
                        sc[:, hi * 512:(hi + 1) * 512],
                        lhsT=kt_sb[hp][hi * 64:(hi + 1) * 64,
                                       kt * 128:(kt + 1) * 128],
                        rhs=qt_sb[hp][hi * 64:(hi + 1) * 64,
                                      qc * 512:(qc + 1) * 512],
                        start=True, stop=True,
                    )
                # drain the PV backlog inside the final step so the tail
                # holds only the very last PV pair
                lim = 1 if (gi == 3 and kt == NKT - 1) else 2
                while len(pend) >= lim:
                    flush_pv(pend.popleft())
                # filler pacing: skip the step after a group boundary (the
                # accumulator drain copies must clear the DVE promptly);
                # pop 2 late in the last group so stage C of group 2 (and
                # the qc0 store) lands before the tail.
                if gi > 0 and kt in (0, 1):
                    n_pop = 0
                elif gi == 3 and kt >= 2:
                    n_pop = 2
                else:
                    n_pop = 1
                for _ in range(n_pop):
                    if fillers:
                        f = fillers.popleft()
                        f()
                pt = pt_p.tile([128, 1024], BF16, tag="pt",
                               name=f"pt{gi}_{kt}")
                if gi == 3 and kt == NKT - 1:
                    # final step: exp + PV + drain in q-halves so the tail
                    # store chain starts half an exp earlier
                    ptv = pt.rearrange("p (h q) -> p h q", h=2)
                    scv = sc.rearrange("p (h q) -> p h q", h=2)
                    entry = (pt, kt, accs, hp, qc, True)
                    for half in range(2):
                        nc.scalar.activation(
                            out=ptv[:, :, half * 256:(half + 1) * 256],
                            in_=scv[:, :, half * 256:(half + 1) * 256],
                            func=EXPF, scale=0.125)
                        flush_pv(entry, half=half)
                else:
                    nc.scalar.activation(out=pt, in_=sc, func=EXPF,
                                         scale=0.125)
                    pend.append((pt, kt, accs, hp, qc,
                                 gi == len(groups) - 1))

        while pend:
            flush_pv(pend.popleft())
        while fillers:
            fillers.popleft()()

    if not nc.is_finalized():
        nc.finalize()
    return nc


def kernel(query: np.ndarray, key: np.ndarray, value: np.ndarray,
           _trace: bool = False):
    if "nc" not in _CACHE:
        _CACHE["nc"] = _build_program()
    nc = _CACHE["nc"]

    query = np.ascontiguousarray(query, dtype=np.float32)
    key = np.ascontiguousarray(key, dtype=np.float32)
    value = np.ascontiguousarray(value, dtype=np.float32)

    in_maps = []
    for c in range(N_CORES):
        b, g = divmod(c, HPC)
        m = {}
        for hp in range(2):
            cols = slice(g * CW + hp * HW, g * CW + (hp + 1) * HW)
            m[f"q{hp}"] = np.ascontiguousarray(query[b, :, cols])
            m[f"k{hp}"] = np.ascontiguousarray(key[b, :, cols])
            m[f"v{hp}"] = np.ascontiguousarray(value[b, :, cols])
        in_maps.append(m)

    res = run_bass_kernel_spmd(
        nc, in_maps, core_ids=list(range(N_CORES)), trace=_trace
    )
    out = np.empty((B, S, D), dtype=np.float32)
    for c in range(N_CORES):
        b, g = divmod(c, HPC)
        for hp in range(2):
            out[b, :, g * CW + hp * HW:g * CW + (hp + 1) * HW] = \
                res.results[c][f"o{hp}"]
    if _trace:
        _CACHE["last_result"] = res
    return out


# revision 66
# speedup vs baseline: 1.0177x; 1.0177x over previous
"""LowRankAttention Trainium2 kernel (v3: head-pair-split DMA + bf16 stage A).

Math shortcut: scores = Q K^T / 8 per (batch, head) has rank <= d_head = 64,
while the truncated SVD keeps rank min(int(1024*0.1), 256) = 102 > 64, so the
low-rank reconstruction is EXACT and the module reduces to plain softmax
attention. Scores are ~N(0,1) (|s| < 8 for these inputs), so exp without
max-subtraction is fp32-safe; the softmax denominator comes for free from a
ones-column appended to the V weights of the PV matmul.

Sharding: 32 (batch, head) pairs over 8 cores; core c owns batch c//4 and
heads 4*(c%4) .. +4 (d_model cols 256*(c%4) .. +256). No collectives.

Structure (per core; all costs measured from neuron-profile ntff):
- The host splits inputs into per-head-pair [1024,128] tensors. Nine
  loads are chained FIFO on the sync HWDGE ring in consumption order
  (q0 k0a k0b1 v0a k0b2 v0b k1 q1 v1), each in (p t) layout = one
  2-4KB-contiguous descriptor per partition (~21ns/descriptor HWDGE
  processing makes small descriptors the dominant cost otherwise). k0's
  tile-0 slot rides its own tiny DMA so the first scores matmul is not
  gated on a 512KB completion; head-pair-1 data lands ~14 steps before
  it is needed.
- All Q/K transposes ride the bf16 path (DVE pre-cast + 1-cycle/row PE
  transpose). PE HAM warm-up matmuls fill the pre-DMA window (transposes
  don't count as PE-busy for the clock gate).
- Steady state: per kt step the PE runs one row-packed scores pair + two
  lag-2 PV matmuls under a gapless ~1.0us/step ACT exp stream; stage A/C
  work rides a paced filler deque. DVE completion sems are counters, so
  any DVE op waiting on a late DMA gates every later-emitted PE filler:
  DMA-gated casts are placed so they pop only after their data lands,
  and pops are skipped at each group boundary's kt0 so the accumulator
  drains clear the DVE promptly. Three PSUM acc banks (pss 1 + sc 4 +
  ac 3 = 8) give each group's first PV a bank whose drain finished a
  full drain-length before the PV issues.
- Outputs are per-(head-pair, q-chunk) [128,4,128] buffers -> every
  store is 128 x 2KB-contiguous descriptors, fired as soon as its
  group's stage C completes; three stores drain mid-stream. The final
  step runs exp/PV/drain/normalize/store in q-halves across both HWDGE
  rings so the last store issues ~1.7us earlier (the ~3us HBM write
  receipt after the last byte is the kernel's true tail).
"""

import sys

if "/opt/trn_rl_repo" not in sys.path:
    sys.path.insert(0, "/opt/trn_rl_repo")

from collections import deque
from contextlib import ExitStack

import numpy as np

import concourse.bass as bass
import concourse.bacc as bacc
import concourse.tile as tile
from concourse import mybir
from concourse.masks import make_identity
from concourse.bass_utils import run_bass_kernel_spmd

B, S, D = 2, 1024, 1024
H, DH = 16, 64
N_CORES = 8
HPC = 4          # heads per core
CW = HPC * DH    # per-core column width = 256
HW = 128         # head-pair width
FP32 = mybir.dt.float32
BF16 = mybir.dt.bfloat16
EXPF = mybir.ActivationFunctionType.Exp
NKT = 8          # k tiles of 128
N_WARM = 8       # PE HAM warm-up matmuls during the initial DMA wait

_CACHE: dict = {}


def _build_program() -> bass.Bass:
    nc = bacc.Bacc(trn_type="TRN2", num_swdge_queues=1)
    ins = {}
    for hp in range(2):
        for nm in ("q", "k", "v"):
            ins[(nm, hp)] = nc.dram_tensor(f"{nm}{hp}", [S, HW], FP32,
                                           kind="ExternalInput")
    o_d = {hp: nc.dram_tensor(f"o{hp}", [S, HW], FP32, kind="ExternalOutput")
           for hp in range(2)}

    with ExitStack() as ctx:
        tc = ctx.enter_context(tile.TileContext(nc))
        const = ctx.enter_context(tc.tile_pool(name="const", bufs=1))
        raw_p = ctx.enter_context(tc.tile_pool(name="raw", bufs=6))
        natb = ctx.enter_context(tc.tile_pool(name="natb", bufs=4))
        vo_p = ctx.enter_context(tc.tile_pool(name="vo", bufs=4))
        qt_p = ctx.enter_context(tc.tile_pool(name="qt", bufs=2))
        kt_p = ctx.enter_context(tc.tile_pool(name="kt", bufs=2))
        pt_p = ctx.enter_context(tc.tile_pool(name="pt", bufs=4))
        ot_p = ctx.enter_context(tc.tile_pool(name="ot", bufs=4))
        os_p = ctx.enter_context(tc.tile_pool(name="os", bufs=4))
        rc_p = ctx.enter_context(tc.tile_pool(name="rc", bufs=4))
        idb_p = ctx.enter_context(tc.tile_pool(name="idb", bufs=1))
        # PSUM budget: pss 1 + sc 2x2 + ac 3x1 = 8 banks. Three acc banks
        # make each group's first PV land on a bank whose drain finished a
        # full drain-length before the PV issues (no boundary stall).
        pss = ctx.enter_context(tc.tile_pool(name="pss", bufs=1, space="PSUM"))
        sc_p = ctx.enter_context(tc.tile_pool(name="sc", bufs=2, space="PSUM"))
        ac_p = ctx.enter_context(tc.tile_pool(name="ac", bufs=3, space="PSUM"))

        # ---------- input DMA chain: (p t) layout, 4KB/partition
        # descriptors, chained FIFO on sync in consumption order. k0's
        # tile-0 slot rides its own tiny DMA so the critical first scores
        # matmul isn't gated on the full 512KB k0 completion. ----------
        raws = {}

        def load(nm, hp, t0, nt, tag):
            rw = raw_p.tile([128, nt, HW], FP32, tag="raw", name=tag)
            nc.sync.dma_start(
                out=rw,
                in_=ins[(nm, hp)].rearrange("(p t) c -> p t c",
                                            p=128)[:, t0:t0 + nt, :],
            )
            raws[tag] = rw

        load("q", 0, 0, NKT, "q0")
        load("k", 0, 0, 1, "k0a")
        load("k", 0, 1, 3, "k0b1")
        load("v", 0, 0, 2, "v0a")
        load("k", 0, 4, 4, "k0b2")
        load("v", 0, 2, 6, "v0b")
        load("k", 1, 0, NKT, "k1")
        load("q", 1, 0, NKT, "q1")
        load("v", 1, 0, NKT, "v1")

        identb = idb_p.tile([128, 128], BF16)
        make_identity(nc, identb)
        ident = const.tile([128, 128], FP32)
        make_identity(nc, ident)
        # the exp table set loads via the pseudo-instruction walrus attaches
        # to ACT's first activation (the prologue scalar.copy casts below),
        # firing immediately at kernel start under the q0 DMA drain
        # HAM warm-up: real matmuls (transposes don't count as PE-busy)
        wrm = const.tile([128, 512], BF16)
        nc.vector.memset(wrm, 0.0)
        wps = ac_p.tile([128, 512], FP32, tag="ac", name="warmps")
        for _ in range(N_WARM):
            nc.tensor.matmul(wps, lhsT=identb, rhs=wrm, start=True, stop=True)

        # KT/QT: two heads stacked on partitions 0:64 / 64:128 so the d=64
        # scores matmuls row-pack into the PE array as concurrent pairs.
        kt_sb = {hp: kt_p.tile([128, S], BF16, tag="kt", name=f"kt{hp}")
                 for hp in range(2)}
        qt_sb = {hp: qt_p.tile([128, S], BF16, tag="qt", name=f"qt{hp}")
                 for hp in range(2)}

        # split raw tiles: map (nm, hp) -> [(tag, slot0, nslots), ...]
        pieces = {
            ("k", 0): [("k0a", 0, 1), ("k0b1", 1, 3), ("k0b2", 4, 4)],
            ("v", 0): [("v0a", 0, 2), ("v0b", 2, 6)],
            ("q", 0): [("q0", 0, 8)],
            ("k", 1): [("k1", 0, 8)],
            ("q", 1): [("q1", 0, 8)],
            ("v", 1): [("v1", 0, 8)],
        }

        def raw_view(nm, hp, j0, nj):
            """AP view of raw slots j0..j0+nj-1 (must lie in one piece)."""
            for tag, s0, ns in pieces[(nm, hp)]:
                if s0 <= j0 and j0 + nj <= s0 + ns:
                    return raws[tag][:, j0 - s0:j0 - s0 + nj, :]
            raise AssertionError((nm, hp, j0, nj))

        def cast_raw(nm, hp, j0=0, nj=NKT, eng=None):
            """Pre-cast of raw fp32 -> bf16 (tile slots j0:j0+nj). Stream
            casts ride the otherwise-idle GpSimd engine so a cast waiting
            on a late DMA never gates DVE's completion counter."""
            key = ("nb", nm, hp)
            if key not in _CACHE:
                _CACHE[key] = natb.tile([128, NKT, HW], BF16, tag="natb",
                                        name=f"nb{nm}{hp}")
            eng = eng if eng is not None else nc.vector
            if eng is nc.scalar:
                nc.scalar.copy(out=_CACHE[key][:, j0:j0 + nj, :],
                               in_=raw_view(nm, hp, j0, nj))
            else:
                eng.tensor_copy(out=_CACHE[key][:, j0:j0 + nj, :],
                                in_=raw_view(nm, hp, j0, nj))

        def tp_bf16(nm, hp, dst, j0, nj, pool=None):
            """bf16 PE transposes of tiles j0..j0+nj-1 -> dst[:, 128*j]."""
            nb = _CACHE[("nb", nm, hp)]
            pool = pool if pool is not None else pss
            ps = pool.tile([128, 128 * nj], BF16, tag=pool is pss and "pss"
                           or "ac", name=f"ps{nm}{hp}{j0}")
            for j in range(nj):
                nc.tensor.transpose(
                    out=ps[:, j * 128:(j + 1) * 128],
                    in_=nb[:, j0 + j, :], identity=identb)
            nc.vector.tensor_copy(
                out=dst[:, j0 * 128:(j0 + nj) * 128], in_=ps)

        # V weights [V|1]: no transpose needed (V is naturally k-major)
        vos = {}
        for hp in range(2):
            for hi in range(2):
                vo = vo_p.tile([128, NKT, DH + 1], BF16, tag="vo",
                               name=f"vo{hp}{hi}")
                nc.vector.memset(vo[:, :, DH:DH + 1], 1.0)
                vos[(hp, hi)] = vo

        def vo_cast(hp, hi, s0=0, ns=NKT):
            nc.vector.tensor_copy(
                out=vos[(hp, hi)][:, s0:s0 + ns, 0:DH],
                in_=raw_view("v", hp, s0, ns)[:, :, hi * DH:(hi + 1) * DH])

        # output: per-(hp,qc) buffers -> every store is 128 x 2KB-contiguous
        # descriptors and fires as soon as its group's stage C completes
        osb = {(hp, qc): os_p.tile([128, 4, HW], FP32, tag="os",
                                   name=f"osb{hp}{qc}")
               for hp in range(2) for qc in range(2)}
        o_v = {hp: o_d[hp].rearrange("(p t) c -> p t c", p=128)
               for hp in range(2)}

        # ---------- prologue stage A (critical path to the first exp):
        # fine-grained q0 chains (2 tiles per cast/tp/copy hop) while k0's
        # tiny j0 load lands, then the k0 j0 tile ----
        cast_raw("q", 0, 0, 2, eng=nc.vector)
        tp_bf16("q", 0, qt_sb[0], 0, 2)
        # ACT is idle after its table load: run the second q cast and the
        # k0 tile-0 cast there, concurrent with DVE's first cast + copies
        cast_raw("q", 0, 2, 2, eng=nc.scalar)
        tp_bf16("q", 0, qt_sb[0], 2, 2)
        cast_raw("k", 0, 0, 1, eng=nc.scalar)
        tp_bf16("k", 0, kt_sb[0], 0, 1, pool=ac_p)

        # ---------- filler deque (popped ~1 per kt step). DVE completion
        # sems are counters, so a DVE op waiting on a late DMA gates every
        # later-emitted PE filler: hp1 casts sit in group-1 slots (their
        # DMAs land during group 0) with no-op pads before the v1 casts.
        fillers: deque = deque()
        fillers.append(lambda: (cast_raw("k", 0, 1, 3, eng=nc.vector),
                                tp_bf16("k", 0, kt_sb[0], 1, 3)))   # g0k0
        fillers.append(lambda: (vo_cast(0, 0, 0, 2),
                                vo_cast(0, 1, 0, 2)))               # g0k1
        fillers.append(lambda: (vo_cast(0, 0, 2, 2),
                                vo_cast(0, 1, 2, 2)))               # g0k2
        fillers.append(lambda: (cast_raw("k", 0, 4, 4),
                                tp_bf16("k", 0, kt_sb[0], 4, 4)))   # g0k3
        fillers.append(lambda: (vo_cast(0, 0, 4, 4),
                                vo_cast(0, 1, 4, 4)))               # g0k4
        fillers.append(lambda: (cast_raw("q", 0, 4, 4),
                                tp_bf16("q", 0, qt_sb[0], 4, 4)))   # g0k5
        fillers.append(lambda: vo_cast(1, 0))                       # g0k6
        fillers.append(lambda: vo_cast(1, 1))                       # g0k7
        fillers.append(lambda: cast_raw("k", 1))                    # g1k2
        fillers.append(lambda: tp_bf16("k", 1, kt_sb[1], 0, 4))     # g1k3
        fillers.append(lambda: tp_bf16("k", 1, kt_sb[1], 4, 4))     # g1k4
        fillers.append(lambda: cast_raw("q", 1))                    # g1k5
        fillers.append(lambda: tp_bf16("q", 1, qt_sb[1], 0, 4))     # g1k6
        fillers.append(lambda: tp_bf16("q", 1, qt_sb[1], 4, 4))     # g1k7

        def emit_acc_drain(hp, qc, accs, last=False):
            """PSUM->SBUF copies freeing the accumulator banks, emitted
            eagerly when a group's last PV retires. For the final group ACT
            is idle, so one copy runs there concurrently."""
            if last:
                # tail: everything is emitted at once, so pipeline in
                # q-halves: drain a-halves first (ACT: hi0, DVE: hi1) so
                # transposes/normalize/store of the a-half overlap the
                # b-half drains
                for hi in range(2):
                    oT = ot_p.tile([DH + 1, 512], FP32, tag="ot",
                                   name=f"oTl{hi}")
                    _CACHE[("c", hp, qc, hi)] = oT
                for half in range(2):
                    sl = slice(half * 256, (half + 1) * 256)
                    nc.scalar.copy(out=_CACHE[("c", hp, qc, 0)][:, sl],
                                   in_=accs[0][:, sl])
                    nc.vector.tensor_copy(
                        out=_CACHE[("c", hp, qc, 1)][:, sl],
                        in_=accs[1][:, sl])
                return
            for hi in range(2):
                oT = ot_p.tile([DH + 1, 512], FP32, tag="ot",
                               name=f"oT{hp}{qc}{hi}")
                _CACHE[("c", hp, qc, hi)] = oT
                nc.vector.tensor_copy(out=oT, in_=accs[hi])

        def tail_stage_c(hp, qc):
            """Last group: half-pipelined stage C so the first store issues
            while the b-half still normalizes."""
            trs = {}
            trs[0] = pss.tile([128, 4, DH + 1], FP32, tag="pss", name="trl0")
            trs[1] = ac_p.tile([128, 4, DH + 1], FP32, tag="ac", name="trl1")
            dst = o_v[hp][:, qc * 4:(qc + 1) * 4, :]
            for half in range(2):
                for hi in range(2):
                    oT = _CACHE[("c", hp, qc, hi)]
                    for qt in range(half * 2, half * 2 + 2):
                        nc.tensor.transpose(
                            out=trs[hi][:, qt, :],
                            in_=oT[:, qt * 128:(qt + 1) * 128],
                            identity=ident[0:DH + 1, 0:DH + 1])
                for hi in range(2):
                    tr = trs[hi]
                    sl = slice(half * 2, half * 2 + 2)
                    r2 = rc_p.tile([128, 2], FP32, tag="rc",
                                   name=f"r2l{half}{hi}")
                    nc.vector.reciprocal(out=r2, in_=tr[:, sl, DH:DH + 1])
                    r2b = bass.AP(tensor=r2.tensor, offset=r2.offset,
                                  ap=[r2.ap[0], [1, 2], [0, DH]])
                    nc.vector.tensor_tensor(
                        out=osb[(hp, qc)][:, sl, hi * DH:(hi + 1) * DH],
                        in0=tr[:, sl, 0:DH], in1=r2b,
                        op=mybir.AluOpType.mult)
                eng = nc.scalar if half == 0 else nc.sync
                eng.dma_start(out=dst[:, half * 2:half * 2 + 2, :],
                              in_=osb[(hp, qc)][:, half * 2:half * 2 + 2, :])

        def stage_c_steps(hp, qc, last=False):
            """Per hi: [2 transposes (tr alloc)], [2 transposes], [normalize
            + maybe store]. tr tiles are allocated inside the closures so the
            pss pool rotation matches pop order."""
            if last:
                return [lambda: tail_stage_c(hp, qc)]
            steps = []

            def c_tp_a(hi):
                oT = _CACHE[("c", hp, qc, hi)]
                # tail: hi1's transpose PSUM comes from the (drained) acc
                # pool so it doesn't serialize behind hi0's in the 1-buf
                # pss pool
                pool, tg = (ac_p, "ac") if (last and hi == 1) else (pss,
                                                                    "pss")
                tr = pool.tile([128, 4, DH + 1], FP32, tag=tg,
                               name=f"tr{hp}{qc}{hi}")
                _CACHE[("tr", hp, qc, hi)] = tr
                for qt in range(2):
                    nc.tensor.transpose(
                        out=tr[:, qt, :],
                        in_=oT[:, qt * 128:(qt + 1) * 128],
                        identity=ident[0:DH + 1, 0:DH + 1])

            def c_tp_b(hi):
                oT = _CACHE[("c", hp, qc, hi)]
                tr = _CACHE[("tr", hp, qc, hi)]
                for qt in range(2, 4):
                    nc.tensor.transpose(
                        out=tr[:, qt, :],
                        in_=oT[:, qt * 128:(qt + 1) * 128],
                        identity=ident[0:DH + 1, 0:DH + 1])

            def c_norm(hi):
                tr = _CACHE[("tr", hp, qc, hi)]
                r4 = rc_p.tile([128, 4], FP32, tag="rc",
                               name=f"r4{hp}{qc}{hi}")
                nc.vector.reciprocal(out=r4, in_=tr[:, :, DH:DH + 1])
                r4b = bass.AP(tensor=r4.tensor, offset=r4.offset,
                              ap=[r4.ap[0], [1, 4], [0, DH]])
                nc.vector.tensor_tensor(
                    out=osb[(hp, qc)][:, :, hi * DH:(hi + 1) * DH],
                    in0=tr[:, :, 0:DH], in1=r4b, op=mybir.AluOpType.mult)
                if hi == 1:
                    dst = o_v[hp][:, qc * 4:(qc + 1) * 4, :]
                    src = osb[(hp, qc)]
                    if last:
                        # tail: split across both HWDGE rings
                        nc.scalar.dma_start(out=dst[:, 0:2, :],
                                            in_=src[:, 0:2, :])
                        nc.sync.dma_start(out=dst[:, 2:4, :],
                                          in_=src[:, 2:4, :])
                    else:
                        nc.sync.dma_start(out=dst, in_=src)

            for hi in range(2):
                steps.append(lambda hi=hi: c_tp_a(hi))
                steps.append(lambda hi=hi: c_tp_b(hi))
                steps.append(lambda hi=hi: c_norm(hi))
            return steps

        # flat 32-step pipeline: PV trails the exp stream by TWO steps and
        # crosses group boundaries, so the in-order PE chain always runs a
        # full step ahead of the ACT exp stream.
        groups = [(0, 0), (0, 1), (1, 0), (1, 1)]  # (hp, qc)
        pend = deque()  # (pt, kt, accs, hp, qc, last)

        def flush_pv(entry, half=None):
            ppt, pkt, accs_, hp_, qc_, last_ = entry
            lo, w = (0, 512) if half is None else (half * 256, 256)
            for hi in range(2):
                nc.tensor.matmul(
                    accs_[hi][:, lo:lo + w],
                    lhsT=vos[(hp_, hi)][:, pkt, :],
                    rhs=ppt[:, hi * 512 + lo:hi * 512 + lo + w],
                    start=(pkt == 0), stop=(pkt == NKT - 1),
                )
            if pkt == NKT - 1 and (half is None or half == 1):
                emit_acc_drain(hp_, qc_, accs_, last=last_)
                fillers.extend(stage_c_steps(hp_, qc_, last=last_))

        for gi, (hp, qc) in enumerate(groups):
            accs = [ac_p.tile([DH + 1, 512], FP32, tag="ac",
                              name=f"acc{hp}_{qc}_{i}") for i in range(2)]
            for kt in range(NKT):
                sc = sc_p.tile([128, 1024], FP32, tag="sc",
                               name=f"sc{gi}_{kt}")
                for hi in range(2):
                    nc.tensor.matmul(
                        sc[:, hi * 512:(hi + 1) * 512],
                        lhsT=kt_sb[hp][hi * 64:(hi + 1) * 64,
                                       kt * 128:(kt + 1) * 128],
                        rhs=qt_sb[hp][hi * 64:(hi + 1) * 64,
                                      qc * 512:(qc + 1) * 512],
                        start=True, stop=True,
                    )
                # drain the PV backlog inside the final step so the tail
                # holds only the very last PV pair
                lim = 1 if (gi == 3 and kt == NKT - 1) else 2
                while len(pend) >= lim:
                    flush_pv(pend.popleft())
                # filler pacing: skip the step after a group boundary (the
                # accumulator drain copies must clear the DVE promptly);
                # pop 2 late in the last group so stage C of group 2 (and
                # the qc0 store) lands before the tail.
                if gi > 0 and kt in (0, 1):
                    n_pop = 0
                elif gi == 3 and kt >= 2:
                    n_pop = 2
                else:
                    n_pop = 1
                for _ in range(n_pop):
                    if fillers:
                        f = fillers.popleft()
                        f()
                pt = pt_p.tile([128, 1024], BF16, tag="pt",
                               name=f"pt{gi}_{kt}")
                if gi == 3 and kt == NKT - 1:
                    # final step: exp + PV + drain in q-halves so the tail
                    # store chain starts half an exp earlier
                    ptv = pt.rearrange("p (h q) -> p h q", h=2)
                    scv = sc.rearrange("p (h q) -> p h q", h=2)
                    entry = (pt, kt, accs, hp, qc, True)
                    for half in range(2):
                        nc.scalar.activation(
                            out=ptv[:, :, half * 256:(half + 1) * 256],
                            in_=scv[:, :, half * 256:(half + 1) * 256],
                            func=EXPF, scale=0.125)
                        flush_pv(entry, half=half)
                else:
                    nc.scalar.activation(out=pt, in_=sc, func=EXPF,
                                         scale=0.125)
                    pend.append((pt, kt, accs, hp, qc,
                                 gi == len(groups) - 1))

        while pend:
            flush_pv(pend.popleft())
        while fillers:
            fillers.popleft()()

    if not nc.is_finalized():
        nc.finalize()
    return nc


def kernel(query: np.ndarray, key: np.ndarray, value: np.ndarray,
           _trace: bool = False):
    if "nc" not in _CACHE:
        _CACHE["nc"] = _build_program()
    nc = _CACHE["nc"]

    query = np.ascontiguousarray(query, dtype=np.float32)
    key = np.ascontiguousarray(key, dtype=np.float32)
    value = np.ascontiguousarray(value, dtype=np.float32)

    in_maps = []
    for c in range(N_CORES):
        b, g = divmod(c, HPC)
        m = {}
        for hp in range(2):
            cols = slice(g * CW + hp * HW, g * CW + (hp + 1) * HW)
            m[f"q{hp}"] = np.ascontiguousarray(query[b, :, cols])
            m[f"k{hp}"] = np.ascontiguousarray(key[b, :, cols])
            m[f"v{hp}"] = np.ascontiguousarray(value[b, :, cols])
        in_maps.append(m)

    res = run_bass_kernel_spmd(
        nc, in_maps, core_ids=list(range(N_CORES)), trace=_trace
    )
    out = np.empty((B, S, D), dtype=np.float32)
    for c in range(N_CORES):
        b, g = divmod(c, HPC)
        for hp in range(2):
            out[b, :, g * CW + hp * HW:g * CW + (hp + 1) * HW] = \
                res.results[c][f"o{hp}"]
    if _trace:
        _CACHE["last_result"] = res
    return out
